# revision 1
# baseline (speedup 1.0000x reference)
"""Trainium2 Bass kernel for 3-layer CuGraphSAGE on a fanout-8 sampled tree.

The sampled graph produced by fanout-based neighbor sampling is a forest of
B=4096 independent trees (children of parent p are rows [4096+8p, 4096+8p+8)).
We shard by seed block: core c gets 512 seeds plus their full 3-hop subtrees
(4 contiguous row blocks of x, exactly 1/8 of all rows, zero halo).

Per-core pipeline (all activations channel-major [128ch, rows] so the matmul
contraction dim is always the partition dim — no transposes on device):
  mean-aggregation = 8 accumulating matmuls with stride-8 rhs APs, the 1/8
  folded into the aggregation weight; self term = 1 more matmul into the same
  PSUM bank; bias+ReLU on ScalarE evicts PSUM->SBUF. h1/h2 live entirely in
  SBUF; only x is streamed from HBM (153.6 MB/core) and 2.25 MB stored.
"""

import os
import numpy as np

# ---------------------------------------------------------------- constants
N_CORES = 8
C = 128                       # channels
B = 4096                      # seeds
S = B // N_CORES              # 512 seeds per core
BLK = [512, 4096, 32768, 262144]          # per-core rows per hop
OFF = [0, 4096, 36864, 299008]            # global start row of each hop block
NLOC = sum(BLK)                           # 299520 local rows
NPAR0 = BLK[0] + BLK[1] + BLK[2]          # 37376 local layer-0 parents
NPAR1 = BLK[0] + BLK[1]                   # 4608 local layer-1 parents
PT = 512                                  # parents per PSUM tile
N_FULL = 2396160
E_FULL = 2392064
OUT_ROWS = 36864

TRACE = os.environ.get("GNN_TRACE", "0") == "1"
DTYPE = os.environ.get("GNN_DTYPE", "float32")
# aggregation path: "dve" = VectorE group-reduce + 1 matmul (best for f32,
# where matmul streams at 1/4 rate); "pe" = 8 accumulating matmuls with
# stride-8 rhs (best for 16-bit dtypes)
AGG = os.environ.get("GNN_AGG", "dve")
LAST_RESULT = None

_BASS_CACHE = {}


def _build_bass(dtype_str, agg):
    import concourse.mybir as mybir
    from concourse import bacc
    from concourse.tile import TileContext

    dt = getattr(mybir.dt, dtype_str)
    f32 = mybir.dt.float32
    Relu = mybir.ActivationFunctionType.Relu
    AxX = mybir.AxisListType.X

    # Bacc (not raw Bass): its compile() pipeline splits multi-sem sync
    # waits into event semaphores — TRN2 allows at most 1 wait/instruction.
    nc = bacc.Bacc()
    xT = nc.dram_tensor("xT", [C, NLOC], dt, kind="ExternalInput")
    # all six 128x128 weight blocks packed into one tensor -> one DMA ->
    # one semaphore lane (per-instruction sync-wait slots are scarce)
    wconsts = nc.dram_tensor("wconsts", [C, 6 * C], dt, kind="ExternalInput")
    bconsts = nc.dram_tensor("bconsts", [C, 3], f32, kind="ExternalInput")
    out = nc.dram_tensor("out", [C, NPAR1], f32, kind="ExternalOutput")
    WIDX = {k: i for i, k in
            enumerate(("w1a", "w1b", "w2a", "w2b", "w3a", "w3b"))}

    with TileContext(nc) as tc:
        with tc.tile_pool(name="const", bufs=1) as constp, \
             tc.tile_pool(name="keep", bufs=1) as keepp, \
             tc.tile_pool(name="cbuf", bufs=2) as cpool, \
             tc.tile_pool(name="dbuf", bufs=3) as dpool, \
             tc.tile_pool(name="hbuf", bufs=2) as hpool, \
             tc.tile_pool(name="obuf", bufs=2) as opool, \
             tc.tile_pool(name="aggbuf", bufs=4) as aggp, \
             tc.tile_pool(name="ps", bufs=6, space="PSUM") as pp:

            wtile = constp.tile([C, 6 * C], dt, name="wtile")
            nc.sync.dma_start(wtile[:, :], wconsts[:, :])
            btile = constp.tile([C, 3], f32, name="btile")
            nc.sync.dma_start(btile[:, :], bconsts[:, :])
            w = {k: wtile[:, C * i: C * (i + 1)] for k, i in WIDX.items()}
            bt = {f"b{i+1}": btile[:, i: i + 1] for i in range(3)}

            xA01 = keepp.tile([C, NPAR1], dt, tag="xA01")
            nc.sync.dma_start(xA01[:, :], xT[:, 0:NPAR1])
            h1self = keepp.tile([C, NPAR1], dt, tag="h1self")
            h2sb = keepp.tile([C, NPAR1], dt, tag="h2sb")

            def sage_tile(psum, wa, wb, children_ap, self_ap):
                # psum[o, p] = sum_e (W_a/8)[o,:] @ children[:, 8p+e]
                #            +  W_b[o,:] @ self[:, p]
                if agg == "pe":
                    cv = children_ap.rearrange("c (p e) -> c p e", e=8)
                    for e in range(8):
                        nc.tensor.matmul(psum, w[wa], cv[:, :, e],
                                         start=(e == 0), stop=False)
                else:
                    # group-sum the 8 siblings on VectorE (stride-1 inner
                    # reduce), then contract once on the TensorEngine --
                    # fp32 matmul streams at 1/4 rate, so 8x fewer matmuls
                    # wins even though DVE reduce is 1 elem/cycle/lane.
                    aggt = aggp.tile([C, PT], dt, tag="agg", name="aggt")
                    nc.vector.reduce_sum(
                        aggt[:, :],
                        children_ap.rearrange("c (p e) -> c p e", e=8),
                        axis=AxX)
                    nc.tensor.matmul(psum, w[wa], aggt[:, :],
                                     start=True, stop=False)
                nc.tensor.matmul(psum, w[wb], self_ap,
                                 start=False, stop=True)

            n_t = NPAR1 // PT                    # 9 outer tiles
            for t in range(n_t):
                # x rows [512+4096t, 512+4096(t+1)): children of layer-0
                # parents [512t, 512(t+1)) AND (for t>=1) self-features of
                # layer-0 parents [512+4096t, ...).
                Ct = cpool.tile([C, 8 * PT], dt, tag="C")
                nc.sync.dma_start(Ct[:, :],
                                  xT[:, S + 8 * PT * t: S + 8 * PT * (t + 1)])

                # layer-0 tile -> h1self[:, 512t:512(t+1)]
                ps0 = pp.tile([C, PT], f32, tag="ps")
                sage_tile(ps0, "w1a", "w1b", Ct[:, :],
                          xA01[:, PT * t: PT * (t + 1)])
                nc.scalar.activation(h1self[:, PT * t: PT * (t + 1)], ps0,
                                     Relu, bias=bt["b1"])

                if t == 0:
                    # parents [512, 4608) already land in h1self via the
                    # ps0 tiles t=1..8; layer-1 tile 0 is emitted after the
                    # loop reading h1self directly (no duplicate compute or
                    # duplicate HBM read of their children).
                    continue

                # 8 layer-0 tiles for parents [512+4096t, 512+4096(t+1))
                h1tmp = hpool.tile([C, 8 * PT], dt, tag="h1tmp")
                for u in range(8):
                    base = NPAR1 + 8 * PT * (8 * t + u)
                    D = dpool.tile([C, 8 * PT], dt, tag="D")
                    nc.sync.dma_start(D[:, :], xT[:, base: base + 8 * PT])
                    psu = pp.tile([C, PT], f32, tag="ps")
                    sage_tile(psu, "w1a", "w1b", D[:, :],
                              Ct[:, PT * u: PT * (u + 1)])
                    nc.scalar.activation(h1tmp[:, PT * u: PT * (u + 1)], psu,
                                         Relu, bias=bt["b1"])

                # layer-1 tile for parents [512t, 512(t+1)) -> h2
                ps1 = pp.tile([C, PT], f32, tag="ps")
                sage_tile(ps1, "w2a", "w2b", h1tmp[:, :],
                          h1self[:, PT * t: PT * (t + 1)])
                nc.scalar.activation(h2sb[:, PT * t: PT * (t + 1)], ps1,
                                     Relu, bias=bt["b2"])

            # layer-1 tile 0: children h1[512:4608) = h1self slice
            ps1z = pp.tile([C, PT], f32, tag="ps")
            sage_tile(ps1z, "w2a", "w2b", h1self[:, S:NPAR1],
                      h1self[:, 0:S])
            nc.scalar.activation(h2sb[:, 0:S], ps1z, Relu, bias=bt["b2"])

            # layer 2: parents [0, 512) aggregate h2[512:4608); rows
            # [512, 4608) have no in-edges (agg = 0) -> self term only.
            ps2 = pp.tile([C, PT], f32, tag="ps")
            sage_tile(ps2, "w3a", "w3b", h2sb[:, S:NPAR1], h2sb[:, 0:S])
            o0 = opool.tile([C, PT], f32, tag="o")
            nc.scalar.activation(o0[:, :], ps2, Relu, bias=bt["b3"])
            nc.sync.dma_start(out[:, 0:S], o0[:, :])
            for t in range(1, n_t):
                psn = pp.tile([C, PT], f32, tag="ps")
                nc.tensor.matmul(psn, w["w3b"],
                                 h2sb[:, PT * t: PT * (t + 1)],
                                 start=True, stop=True)
                on = opool.tile([C, PT], f32, tag="o")
                nc.scalar.activation(on[:, :], psn, Relu, bias=bt["b3"])
                nc.sync.dma_start(out[:, PT * t: PT * (t + 1)], on[:, :])

    nc.compile()
    return nc


def _get_bass(dtype_str, agg="dve"):
    key = (dtype_str, agg)
    if key not in _BASS_CACHE:
        _BASS_CACHE[key] = _build_bass(dtype_str, agg)
    return _BASS_CACHE[key]


def _edge_is_tree(edge):
    if edge.shape != (2, E_FULL):
        return False
    ar = np.arange(E_FULL, dtype=np.int64)
    return (np.array_equal(edge[0], (B + ar).astype(np.int32))
            and np.array_equal(edge[1], (ar // 8).astype(np.int32)))


def _fallback(x, edge, W1, b1, W2, b2, W3, b3):
    # General (structure-agnostic) CPU implementation; only used if the
    # inputs are not the fanout-8 tree this kernel is specialized for.
    sizes = [(N_FULL, E_FULL), (299008, 294912), (36864, 32768)]
    params = [(W1, b1), (W2, b2), (W3, b3)]
    x = x.astype(np.float32)
    for (n, e), (Wl, bl) in zip(sizes, params):
        src = edge[0, :e].astype(np.int64)
        dst = edge[1, :e].astype(np.int64)
        x = x[:n]
        agg = np.zeros((n, x.shape[1]), np.float32)
        np.add.at(agg, dst, x[src])
        deg = np.bincount(dst, minlength=n).astype(np.float32)
        agg /= np.maximum(deg, 1.0)[:, None]
        x = np.maximum(np.concatenate([agg, x], axis=1) @ Wl.T + bl, 0.0)
    return x


def kernel(**inputs):
    global LAST_RESULT
    x = np.asarray(inputs["x"])
    edge = np.asarray(inputs["edge"])
    W = [np.asarray(inputs[k], dtype=np.float32) for k in ("W1", "W2", "W3")]
    bias = [np.asarray(inputs[k], dtype=np.float32) for k in ("b1", "b2", "b3")]

    if x.shape != (N_FULL, C) or not _edge_is_tree(edge):
        return _fallback(x, edge, W[0], bias[0], W[1], bias[1], W[2], bias[2])

    from concourse.bass_utils import run_bass_kernel_spmd

    if DTYPE == "bfloat16":
        np_dt = _bf16()
    else:
        np_dt = {"float32": np.float32, "float32r": np.float32,
                 "float16": np.float16}[DTYPE]
    x = np.ascontiguousarray(x, dtype=np.float32)

    wblocks = []
    for li in range(3):
        wblocks.append((W[li][:, :C] / 8.0).T)     # agg part, mean folded in
        wblocks.append(W[li][:, C:].T)             # self part
    wconsts = np.ascontiguousarray(np.concatenate(wblocks, axis=1)).astype(np_dt)
    bconsts = np.ascontiguousarray(np.stack(bias, axis=1))      # [128, 3] f32

    in_maps = []
    for c in range(N_CORES):
        xloc = np.concatenate(
            [x[OFF[h] + BLK[h] * c: OFF[h] + BLK[h] * (c + 1)] for h in range(4)],
            axis=0)
        xTc = np.ascontiguousarray(xloc.T).astype(np_dt, copy=False)
        in_maps.append({"xT": xTc, "wconsts": wconsts, "bconsts": bconsts})

    nc = _get_bass(DTYPE, AGG)
    res = run_bass_kernel_spmd(nc, in_maps, list(range(N_CORES)), trace=TRACE)
    LAST_RESULT = res

    out = np.empty((OUT_ROWS, C), np.float32)
    for c in range(N_CORES):
        oc = np.asarray(res.results[c]["out"])
        out[S * c: S * (c + 1)] = oc[:, :S].T
        out[B + 8 * S * c: B + 8 * S * (c + 1)] = oc[:, S:].T
    return out


def _bf16():
    import ml_dtypes
    return ml_dtypes.bfloat16



# revision 2
# speedup vs baseline: 1.2207x; 1.2207x over previous
"""Trainium2 Bass kernel for 3-layer CuGraphSAGE on a fanout-8 sampled tree.

The sampled graph produced by fanout-based neighbor sampling is a forest of
B=4096 independent trees (children of parent p are rows [4096+8p, 4096+8p+8)).
We shard by seed block: core c gets 512 seeds plus their full 3-hop subtrees
(4 contiguous row blocks of x, exactly 1/8 of all rows, zero halo).

Per-core pipeline (all activations channel-major [128ch, rows] so the matmul
contraction dim is always the partition dim — no transposes on device):
  mean-aggregation = 8 accumulating matmuls with stride-8 rhs APs, the 1/8
  folded into the aggregation weight; self term = 1 more matmul into the same
  PSUM bank; bias+ReLU on ScalarE evicts PSUM->SBUF. h1/h2 live entirely in
  SBUF; only x is streamed from HBM (153.6 MB/core) and 2.25 MB stored.
"""

import os
import numpy as np

# ---------------------------------------------------------------- constants
N_CORES = 8
C = 128                       # channels
B = 4096                      # seeds
S = B // N_CORES              # 512 seeds per core
BLK = [512, 4096, 32768, 262144]          # per-core rows per hop
OFF = [0, 4096, 36864, 299008]            # global start row of each hop block
NLOC = sum(BLK)                           # 299520 local rows
NPAR0 = BLK[0] + BLK[1] + BLK[2]          # 37376 local layer-0 parents
NPAR1 = BLK[0] + BLK[1]                   # 4608 local layer-1 parents
PT = 512                                  # parents per PSUM tile
N_FULL = 2396160
E_FULL = 2392064
OUT_ROWS = 36864

TRACE = os.environ.get("GNN_TRACE", "0") == "1"
DTYPE = os.environ.get("GNN_DTYPE", "float32")
# aggregation path: "dve" = VectorE group-reduce + 1 matmul (best for f32,
# where matmul streams at 1/4 rate); "pe" = 8 accumulating matmuls with
# stride-8 rhs (best for 16-bit dtypes)
AGG = os.environ.get("GNN_AGG", "dve")
LAST_RESULT = None

_BASS_CACHE = {}


def _build_bass(dtype_str, agg):
    import concourse.mybir as mybir
    from concourse import bacc
    from concourse.tile import TileContext

    dt = getattr(mybir.dt, dtype_str)
    f32 = mybir.dt.float32
    Relu = mybir.ActivationFunctionType.Relu
    AxX = mybir.AxisListType.X

    # Bacc (not raw Bass): its compile() pipeline splits multi-sem sync
    # waits into event semaphores — TRN2 allows at most 1 wait/instruction.
    nc = bacc.Bacc()
    xT = nc.dram_tensor("xT", [C, NLOC], dt, kind="ExternalInput")
    # all six 128x128 weight blocks packed into one tensor -> one DMA ->
    # one semaphore lane (per-instruction sync-wait slots are scarce)
    wconsts = nc.dram_tensor("wconsts", [C, 6 * C], dt, kind="ExternalInput")
    bconsts = nc.dram_tensor("bconsts", [C, 3], f32, kind="ExternalInput")
    out = nc.dram_tensor("out", [C, NPAR1], f32, kind="ExternalOutput")
    WIDX = {k: i for i, k in
            enumerate(("w1a", "w1b", "w2a", "w2b", "w3a", "w3b"))}

    with TileContext(nc) as tc:
        with tc.tile_pool(name="const", bufs=1) as constp, \
             tc.tile_pool(name="keep", bufs=1) as keepp, \
             tc.tile_pool(name="cbuf", bufs=2) as cpool, \
             tc.tile_pool(name="dbuf", bufs=3) as dpool, \
             tc.tile_pool(name="hbuf", bufs=2) as hpool, \
             tc.tile_pool(name="obuf", bufs=2) as opool, \
             tc.tile_pool(name="aggbuf", bufs=4) as aggp, \
             tc.tile_pool(name="ps", bufs=6, space="PSUM") as pp:

            wtile = constp.tile([C, 6 * C], dt, name="wtile")
            nc.sync.dma_start(wtile[:, :], wconsts[:, :])
            btile = constp.tile([C, 3], f32, name="btile")
            nc.sync.dma_start(btile[:, :], bconsts[:, :])
            w = {k: wtile[:, C * i: C * (i + 1)] for k, i in WIDX.items()}
            bt = {f"b{i+1}": btile[:, i: i + 1] for i in range(3)}

            xA01 = keepp.tile([C, NPAR1], dt, tag="xA01")
            nc.sync.dma_start(xA01[:, :], xT[:, 0:NPAR1])
            h1self = keepp.tile([C, NPAR1], dt, tag="h1self")
            h2sb = keepp.tile([C, NPAR1], dt, tag="h2sb")

            def sage_tile(psum, wa, wb, children_ap, self_ap):
                # psum[o, p] = sum_e (W_a/8)[o,:] @ children[:, 8p+e]
                #            +  W_b[o,:] @ self[:, p]
                if agg == "pe":
                    cv = children_ap.rearrange("c (p e) -> c p e", e=8)
                    for e in range(8):
                        nc.tensor.matmul(psum, w[wa], cv[:, :, e],
                                         start=(e == 0), stop=False)
                else:
                    # group-sum the 8 siblings on VectorE (stride-1 inner
                    # reduce), then contract once on the TensorEngine --
                    # fp32 matmul streams at 1/4 rate, so 8x fewer matmuls
                    # wins even though DVE reduce is 1 elem/cycle/lane.
                    aggt = aggp.tile([C, PT], dt, tag="agg", name="aggt")
                    with nc.allow_low_precision(
                            reason="8-term sibling sum; DVE rounds on write"):
                        nc.vector.reduce_sum(
                            aggt[:, :],
                            children_ap.rearrange("c (p e) -> c p e", e=8),
                            axis=AxX)
                    nc.tensor.matmul(psum, w[wa], aggt[:, :],
                                     start=True, stop=False)
                nc.tensor.matmul(psum, w[wb], self_ap,
                                 start=False, stop=True)

            n_t = NPAR1 // PT                    # 9 outer tiles
            for t in range(n_t):
                # x rows [512+4096t, 512+4096(t+1)): children of layer-0
                # parents [512t, 512(t+1)) AND (for t>=1) self-features of
                # layer-0 parents [512+4096t, ...).
                Ct = cpool.tile([C, 8 * PT], dt, tag="C")
                nc.sync.dma_start(Ct[:, :],
                                  xT[:, S + 8 * PT * t: S + 8 * PT * (t + 1)])

                # layer-0 tile -> h1self[:, 512t:512(t+1)]
                ps0 = pp.tile([C, PT], f32, tag="ps")
                sage_tile(ps0, "w1a", "w1b", Ct[:, :],
                          xA01[:, PT * t: PT * (t + 1)])
                nc.scalar.activation(h1self[:, PT * t: PT * (t + 1)], ps0,
                                     Relu, bias=bt["b1"])

                if t == 0:
                    # parents [512, 4608) already land in h1self via the
                    # ps0 tiles t=1..8; layer-1 tile 0 is emitted after the
                    # loop reading h1self directly (no duplicate compute or
                    # duplicate HBM read of their children).
                    continue

                # 8 layer-0 tiles for parents [512+4096t, 512+4096(t+1))
                h1tmp = hpool.tile([C, 8 * PT], dt, tag="h1tmp")
                for u in range(8):
                    base = NPAR1 + 8 * PT * (8 * t + u)
                    D = dpool.tile([C, 8 * PT], dt, tag="D")
                    nc.sync.dma_start(D[:, :], xT[:, base: base + 8 * PT])
                    psu = pp.tile([C, PT], f32, tag="ps")
                    sage_tile(psu, "w1a", "w1b", D[:, :],
                              Ct[:, PT * u: PT * (u + 1)])
                    nc.scalar.activation(h1tmp[:, PT * u: PT * (u + 1)], psu,
                                         Relu, bias=bt["b1"])

                # layer-1 tile for parents [512t, 512(t+1)) -> h2
                ps1 = pp.tile([C, PT], f32, tag="ps")
                sage_tile(ps1, "w2a", "w2b", h1tmp[:, :],
                          h1self[:, PT * t: PT * (t + 1)])
                nc.scalar.activation(h2sb[:, PT * t: PT * (t + 1)], ps1,
                                     Relu, bias=bt["b2"])

            # layer-1 tile 0: children h1[512:4608) = h1self slice
            ps1z = pp.tile([C, PT], f32, tag="ps")
            sage_tile(ps1z, "w2a", "w2b", h1self[:, S:NPAR1],
                      h1self[:, 0:S])
            nc.scalar.activation(h2sb[:, 0:S], ps1z, Relu, bias=bt["b2"])

            # layer 2: parents [0, 512) aggregate h2[512:4608); rows
            # [512, 4608) have no in-edges (agg = 0) -> self term only.
            ps2 = pp.tile([C, PT], f32, tag="ps")
            sage_tile(ps2, "w3a", "w3b", h2sb[:, S:NPAR1], h2sb[:, 0:S])
            o0 = opool.tile([C, PT], f32, tag="o")
            nc.scalar.activation(o0[:, :], ps2, Relu, bias=bt["b3"])
            nc.sync.dma_start(out[:, 0:S], o0[:, :])
            for t in range(1, n_t):
                psn = pp.tile([C, PT], f32, tag="ps")
                nc.tensor.matmul(psn, w["w3b"],
                                 h2sb[:, PT * t: PT * (t + 1)],
                                 start=True, stop=True)
                on = opool.tile([C, PT], f32, tag="o")
                nc.scalar.activation(on[:, :], psn, Relu, bias=bt["b3"])
                nc.sync.dma_start(out[:, PT * t: PT * (t + 1)], on[:, :])

    nc.compile()
    return nc


def _get_bass(dtype_str, agg="dve"):
    key = (dtype_str, agg)
    if key not in _BASS_CACHE:
        _BASS_CACHE[key] = _build_bass(dtype_str, agg)
    return _BASS_CACHE[key]


def _edge_is_tree(edge):
    if edge.shape != (2, E_FULL):
        return False
    ar = np.arange(E_FULL, dtype=np.int64)
    return (np.array_equal(edge[0], (B + ar).astype(np.int32))
            and np.array_equal(edge[1], (ar // 8).astype(np.int32)))


def _fallback(x, edge, W1, b1, W2, b2, W3, b3):
    # General (structure-agnostic) CPU implementation; only used if the
    # inputs are not the fanout-8 tree this kernel is specialized for.
    sizes = [(N_FULL, E_FULL), (299008, 294912), (36864, 32768)]
    params = [(W1, b1), (W2, b2), (W3, b3)]
    x = x.astype(np.float32)
    for (n, e), (Wl, bl) in zip(sizes, params):
        src = edge[0, :e].astype(np.int64)
        dst = edge[1, :e].astype(np.int64)
        x = x[:n]
        agg = np.zeros((n, x.shape[1]), np.float32)
        np.add.at(agg, dst, x[src])
        deg = np.bincount(dst, minlength=n).astype(np.float32)
        agg /= np.maximum(deg, 1.0)[:, None]
        x = np.maximum(np.concatenate([agg, x], axis=1) @ Wl.T + bl, 0.0)
    return x


def kernel(**inputs):
    global LAST_RESULT
    x = np.asarray(inputs["x"])
    edge = np.asarray(inputs["edge"])
    W = [np.asarray(inputs[k], dtype=np.float32) for k in ("W1", "W2", "W3")]
    bias = [np.asarray(inputs[k], dtype=np.float32) for k in ("b1", "b2", "b3")]

    if x.shape != (N_FULL, C) or not _edge_is_tree(edge):
        return _fallback(x, edge, W[0], bias[0], W[1], bias[1], W[2], bias[2])

    from concourse.bass_utils import run_bass_kernel_spmd

    if DTYPE == "bfloat16":
        np_dt = _bf16()
    else:
        np_dt = {"float32": np.float32, "float32r": np.float32,
                 "float16": np.float16}[DTYPE]
    x = np.ascontiguousarray(x, dtype=np.float32)

    wblocks = []
    for li in range(3):
        wblocks.append((W[li][:, :C] / 8.0).T)     # agg part, mean folded in
        wblocks.append(W[li][:, C:].T)             # self part
    wconsts = np.ascontiguousarray(np.concatenate(wblocks, axis=1)).astype(np_dt)
    bconsts = np.ascontiguousarray(np.stack(bias, axis=1))      # [128, 3] f32

    in_maps = []
    for c in range(N_CORES):
        xloc = np.concatenate(
            [x[OFF[h] + BLK[h] * c: OFF[h] + BLK[h] * (c + 1)] for h in range(4)],
            axis=0)
        xTc = np.ascontiguousarray(xloc.T).astype(np_dt, copy=False)
        in_maps.append({"xT": xTc, "wconsts": wconsts, "bconsts": bconsts})

    nc = _get_bass(DTYPE, AGG)
    res = run_bass_kernel_spmd(nc, in_maps, list(range(N_CORES)), trace=TRACE)
    LAST_RESULT = res

    out = np.empty((OUT_ROWS, C), np.float32)
    for c in range(N_CORES):
        oc = np.asarray(res.results[c]["out"])
        out[S * c: S * (c + 1)] = oc[:, :S].T
        out[B + 8 * S * c: B + 8 * S * (c + 1)] = oc[:, S:].T
    return out


def _bf16():
    import ml_dtypes
    return ml_dtypes.bfloat16



# revision 3
# speedup vs baseline: 1.8184x; 1.4896x over previous
"""Trainium2 Bass kernel for 3-layer CuGraphSAGE on a fanout-8 sampled tree.

The sampled graph is a forest of B=4096 independent trees (children of
parent p are rows [4096+8p, 4096+8p+8)). Shard by seed block: core c gets
512 seeds plus their full 3-hop subtrees (contiguous row blocks, exactly
1/8 of all rows, zero halo, no collectives).

Precision: leaf-hop features (87.5% of bytes) stream as fp8 e3m4, inner
hops as bf16, weights bf16, PSUM accumulation f32. Measured end-to-end
rel err ~4e-3 vs the f32 reference.

Layout: channel-major [128ch, rows] so the matmul contraction dim is the
partition dim. Leaf columns are host-reordered PLANE-major per outer tile
(all sibling-0 cols, then sibling-1, ...) so the mean-aggregation is 8
accumulating matmuls with fully CONTIGUOUS 512-col rhs slabs -- strided
rhs runs 2-4x slower on the PE, contiguous hits 216ns/512col. The 1/8 is
folded into the aggregation weight. Inner aggregations (natural order,
sibling-adjacent) use the DVE reduce (4.4us/tile, dtype-independent); a
few leaf sub-tiles are offloaded to DVE via packed plane-adds to balance
engines. Only x is streamed from HBM (~43 MB/core).
"""

import os
import numpy as np

# ---------------------------------------------------------------- constants
N_CORES = 8
C = 128                       # channels
B = 4096                      # seeds
S = B // N_CORES              # 512 seeds per core
BLK = [512, 4096, 32768, 262144]          # per-core rows per hop
OFF = [0, 4096, 36864, 299008]            # global start row of each hop block
NIN = BLK[0] + BLK[1] + BLK[2]            # 37376 inner rows (hop 0-2)
NLEAF = BLK[3]                            # 262144 leaf rows
NPAR1 = BLK[0] + BLK[1]                   # 4608 layer-1 parents
PT = 512                                  # parents per PSUM tile
LB = 8 * BLK[1]                           # 32768 leaf rows per outer tile
N_FULL = 2396160
E_FULL = 2392064
OUT_ROWS = 36864

TRACE = os.environ.get("GNN_TRACE", "0") == "1"
# number of leaf sub-tiles (of 64) aggregated on DVE instead of PE
NDVE = int(os.environ.get("GNN_NDVE", "8"))
LAST_RESULT = None

_BASS_CACHE = {}


def _leaf_dve_flags(ndve):
    # spread ndve True flags evenly over the 64 leaf sub-tiles
    return [(m * ndve) // 64 != ((m + 1) * ndve) // 64 for m in range(64)]


def _build_bass(ndve):
    import concourse.mybir as mybir
    from concourse import bacc
    from concourse.tile import TileContext

    bf16 = mybir.dt.bfloat16
    f8e3 = mybir.dt.float8e3
    f32 = mybir.dt.float32
    Relu = mybir.ActivationFunctionType.Relu
    AxX = mybir.AxisListType.X
    Add = mybir.AluOpType.add

    dve_flag = _leaf_dve_flags(ndve)

    nc = bacc.Bacc()
    xiT = nc.dram_tensor("xiT", [C, NIN], bf16, kind="ExternalInput")
    xlT = nc.dram_tensor("xlT", [C, NLEAF], f8e3, kind="ExternalInput")
    wconsts = nc.dram_tensor("wconsts", [C, 6 * C], bf16, kind="ExternalInput")
    bconsts = nc.dram_tensor("bconsts", [C, 3], f32, kind="ExternalInput")
    out = nc.dram_tensor("out", [C, NPAR1], f32, kind="ExternalOutput")
    WIDX = {k: i for i, k in
            enumerate(("w1a", "w1b", "w2a", "w2b", "w3a", "w3b"))}

    with TileContext(nc) as tc:
        with tc.tile_pool(name="const", bufs=1) as constp, \
             tc.tile_pool(name="keep", bufs=1) as keepp, \
             tc.tile_pool(name="cbuf", bufs=2) as cpool, \
             tc.tile_pool(name="dbuf", bufs=3) as dpool, \
             tc.tile_pool(name="hbuf", bufs=2) as hpool, \
             tc.tile_pool(name="obuf", bufs=2) as opool, \
             tc.tile_pool(name="aggbuf", bufs=4) as aggp, \
             tc.tile_pool(name="addbuf", bufs=8) as addp, \
             tc.tile_pool(name="ps", bufs=6, space="PSUM") as pp:

            wtile = constp.tile([C, 6 * C], bf16, name="wtile")
            nc.sync.dma_start(wtile[:, :], wconsts[:, :])
            btile = constp.tile([C, 3], f32, name="btile")
            nc.sync.dma_start(btile[:, :], bconsts[:, :])
            w = {k: wtile[:, C * i: C * (i + 1)] for k, i in WIDX.items()}
            bt = {f"b{i+1}": btile[:, i: i + 1] for i in range(3)}

            xA01 = keepp.tile([C, NPAR1], bf16, tag="xA01")
            nc.sync.dma_start(xA01[:, :], xiT[:, 0:NPAR1])
            h1self = keepp.tile([C, NPAR1], bf16, tag="h1self")
            h2sb = keepp.tile([C, NPAR1], bf16, tag="h2sb")

            def dve_sage(psum, wa, wb, children_ap, self_ap):
                # DVE group-reduce (sibling-adjacent natural order) + 2 mms
                aggt = aggp.tile([C, PT], bf16, tag="agg", name="aggt")
                with nc.allow_low_precision(reason="8-term sibling sum"):
                    nc.vector.reduce_sum(
                        aggt[:, :],
                        children_ap.rearrange("c (p e) -> c p e", e=8),
                        axis=AxX)
                nc.tensor.matmul(psum, w[wa], aggt[:, :],
                                 start=True, stop=False)
                nc.tensor.matmul(psum, w[wb], self_ap,
                                 start=False, stop=True)

            def leaf_pe_sage(psum, D, u, self_ap):
                # 8 accumulating mms over contiguous plane slabs
                for e in range(8):
                    nc.tensor.matmul(
                        psum, w["w1a"],
                        D[:, BLK[1] * e + PT * u: BLK[1] * e + PT * (u + 1)],
                        start=(e == 0), stop=False)
                nc.tensor.matmul(psum, w["w1b"], self_ap,
                                 start=False, stop=True)

            def leaf_dve_sage(psum, D, u, self_ap):
                # packed plane-adds: 4x (fp8+fp8->bf16), then 2+1 bf16
                def sl(e):
                    return D[:, BLK[1] * e + PT * u: BLK[1] * e + PT * (u + 1)]
                with nc.allow_low_precision(reason="8-term sibling sum"):
                    t4 = [addp.tile([C, PT], bf16, tag="add", name=f"t4_{j}")
                          for j in range(4)]
                    for j in range(4):
                        nc.vector.tensor_tensor(
                            t4[j][:, :], sl(2 * j), sl(2 * j + 1), op=Add)
                    s0 = addp.tile([C, PT], bf16, tag="add", name="s0")
                    nc.vector.tensor_tensor(s0[:, :], t4[0][:, :],
                                            t4[1][:, :], op=Add)
                    s1 = addp.tile([C, PT], bf16, tag="add", name="s1")
                    nc.vector.tensor_tensor(s1[:, :], t4[2][:, :],
                                            t4[3][:, :], op=Add)
                    aggt = aggp.tile([C, PT], bf16, tag="agg", name="aggd")
                    nc.vector.tensor_tensor(aggt[:, :], s0[:, :],
                                            s1[:, :], op=Add)
                nc.tensor.matmul(psum, w["w1a"], aggt[:, :],
                                 start=True, stop=False)
                nc.tensor.matmul(psum, w["w1b"], self_ap,
                                 start=False, stop=True)

            for t in range(1, 9):
                # inner rows [512+4096t, 512+4096(t+1)): children of layer-0
                # parents [512t, 512(t+1)) and self rows of the 8 leaf tiles
                Ct = cpool.tile([C, 8 * PT], bf16, tag="C")
                nc.sync.dma_start(
                    Ct[:, :], xiT[:, S + 8 * PT * t: S + 8 * PT * (t + 1)])
                # this outer tile's 32768 leaf rows, plane-major
                D = dpool.tile([C, LB], f8e3, tag="D")
                nc.sync.dma_start(D[:, :], xlT[:, LB * (t - 1): LB * t])

                # layer-0 tile for parents [512t, 512(t+1)) (hop-1 nodes)
                ps0 = pp.tile([C, PT], f32, tag="ps", name=f"ps0_{t}")
                dve_sage(ps0, "w1a", "w1b", Ct[:, :],
                         xA01[:, PT * t: PT * (t + 1)])
                nc.scalar.activation(h1self[:, PT * t: PT * (t + 1)], ps0,
                                     Relu, bias=bt["b1"])

                # 8 layer-0 leaf tiles for parents [512+4096t, ...)
                h1tmp = hpool.tile([C, 8 * PT], bf16, tag="h1tmp")
                for u in range(8):
                    psu = pp.tile([C, PT], f32, tag="ps", name=f"psu{t}_{u}")
                    if dve_flag[8 * (t - 1) + u]:
                        leaf_dve_sage(psu, D, u, Ct[:, PT * u: PT * (u + 1)])
                    else:
                        leaf_pe_sage(psu, D, u, Ct[:, PT * u: PT * (u + 1)])
                    nc.scalar.activation(h1tmp[:, PT * u: PT * (u + 1)], psu,
                                         Relu, bias=bt["b1"])

                # layer-1 tile for parents [512t, 512(t+1)) -> h2
                ps1 = pp.tile([C, PT], f32, tag="ps", name=f"ps1_{t}")
                dve_sage(ps1, "w2a", "w2b", h1tmp[:, :],
                         h1self[:, PT * t: PT * (t + 1)])
                nc.scalar.activation(h2sb[:, PT * t: PT * (t + 1)], ps1,
                                     Relu, bias=bt["b2"])

            # layer-0 tile 0 (seeds): children = xA01[512:4608)
            ps0z = pp.tile([C, PT], f32, tag="ps", name="ps0z")
            dve_sage(ps0z, "w1a", "w1b", xA01[:, S:NPAR1], xA01[:, 0:S])
            nc.scalar.activation(h1self[:, 0:S], ps0z, Relu, bias=bt["b1"])

            # layer-1 tile 0: children h1[512:4608) = h1self slice
            ps1z = pp.tile([C, PT], f32, tag="ps", name="ps1z")
            dve_sage(ps1z, "w2a", "w2b", h1self[:, S:NPAR1], h1self[:, 0:S])
            nc.scalar.activation(h2sb[:, 0:S], ps1z, Relu, bias=bt["b2"])

            # layer 2: parents [0, 512) aggregate h2[512:4608); rows
            # [512, 4608) have no in-edges (agg = 0) -> self term only.
            ps2 = pp.tile([C, PT], f32, tag="ps", name="ps2")
            dve_sage(ps2, "w3a", "w3b", h2sb[:, S:NPAR1], h2sb[:, 0:S])
            o0 = opool.tile([C, PT], f32, tag="o", name="o0")
            nc.scalar.activation(o0[:, :], ps2, Relu, bias=bt["b3"])
            nc.sync.dma_start(out[:, 0:S], o0[:, :])
            for t in range(1, 9):
                psn = pp.tile([C, PT], f32, tag="ps", name=f"psn{t}")
                nc.tensor.matmul(psn, w["w3b"],
                                 h2sb[:, PT * t: PT * (t + 1)],
                                 start=True, stop=True)
                on = opool.tile([C, PT], f32, tag="o", name=f"on{t}")
                nc.scalar.activation(on[:, :], psn, Relu, bias=bt["b3"])
                nc.sync.dma_start(out[:, PT * t: PT * (t + 1)], on[:, :])

    nc.compile()
    return nc


def _get_bass(ndve):
    if ndve not in _BASS_CACHE:
        _BASS_CACHE[ndve] = _build_bass(ndve)
    return _BASS_CACHE[ndve]


def _edge_is_tree(edge):
    if edge.shape != (2, E_FULL):
        return False
    ar = np.arange(E_FULL, dtype=np.int64)
    return (np.array_equal(edge[0], (B + ar).astype(np.int32))
            and np.array_equal(edge[1], (ar // 8).astype(np.int32)))


def _fallback(x, edge, W1, b1, W2, b2, W3, b3):
    # General (structure-agnostic) CPU implementation; only used if the
    # inputs are not the fanout-8 tree this kernel is specialized for.
    sizes = [(N_FULL, E_FULL), (299008, 294912), (36864, 32768)]
    params = [(W1, b1), (W2, b2), (W3, b3)]
    x = x.astype(np.float32)
    for (n, e), (Wl, bl) in zip(sizes, params):
        src = edge[0, :e].astype(np.int64)
        dst = edge[1, :e].astype(np.int64)
        x = x[:n]
        agg = np.zeros((n, x.shape[1]), np.float32)
        np.add.at(agg, dst, x[src])
        deg = np.bincount(dst, minlength=n).astype(np.float32)
        agg /= np.maximum(deg, 1.0)[:, None]
        x = np.maximum(np.concatenate([agg, x], axis=1) @ Wl.T + bl, 0.0)
    return x


def kernel(**inputs):
    global LAST_RESULT
    x = np.asarray(inputs["x"])
    edge = np.asarray(inputs["edge"])
    W = [np.asarray(inputs[k], dtype=np.float32) for k in ("W1", "W2", "W3")]
    bias = [np.asarray(inputs[k], dtype=np.float32) for k in ("b1", "b2", "b3")]

    if x.shape != (N_FULL, C) or not _edge_is_tree(edge):
        return _fallback(x, edge, W[0], bias[0], W[1], bias[1], W[2], bias[2])

    import ml_dtypes
    from concourse.bass_utils import run_bass_kernel_spmd

    bf16 = ml_dtypes.bfloat16
    f8e3 = ml_dtypes.float8_e3m4
    x = np.ascontiguousarray(x, dtype=np.float32)

    wblocks = []
    for li in range(3):
        wblocks.append((W[li][:, :C] / 8.0).T)     # agg part, mean folded in
        wblocks.append(W[li][:, C:].T)             # self part
    wconsts = np.concatenate(wblocks, axis=1).astype(bf16)
    wconsts = np.ascontiguousarray(wconsts)
    bconsts = np.ascontiguousarray(np.stack(bias, axis=1))      # [128, 3] f32

    in_maps = []
    for c in range(N_CORES):
        xi = np.concatenate(
            [x[OFF[h] + BLK[h] * c: OFF[h] + BLK[h] * (c + 1)]
             for h in range(3)], axis=0)
        xiTc = np.ascontiguousarray(xi.T).astype(bf16, copy=False)
        xl = x[OFF[3] + BLK[3] * c: OFF[3] + BLK[3] * (c + 1)]
        # plane-major per outer tile: [8 tiles, 4096 parents, 8 sib, 128ch]
        # -> [8, 8 sib, 4096 parents, 128]
        xl = xl.reshape(8, 4096, 8, C).transpose(0, 2, 1, 3).reshape(-1, C)
        xlTc = np.ascontiguousarray(xl.T).astype(f8e3, copy=False)
        in_maps.append({"xiT": xiTc, "xlT": xlTc,
                        "wconsts": wconsts, "bconsts": bconsts})

    nc = _get_bass(NDVE)
    res = run_bass_kernel_spmd(nc, in_maps, list(range(N_CORES)), trace=TRACE)
    LAST_RESULT = res

    out = np.empty((OUT_ROWS, C), np.float32)
    for c in range(N_CORES):
        oc = np.asarray(res.results[c]["out"])
        out[S * c: S * (c + 1)] = oc[:, :S].T
        out[B + 8 * S * c: B + 8 * S * (c + 1)] = oc[:, S:].T
    return out


# revision 4
# speedup vs baseline: 2.7187x; 1.4951x over previous
"""Trainium2 Bass kernel for 3-layer CuGraphSAGE on a fanout-8 sampled tree.

The sampled graph is a forest of B=4096 independent trees (children of
parent p are rows [4096+8p, 4096+8p+8)). Shard by seed block: core c gets
512 seeds plus their full 3-hop subtrees (contiguous row blocks, exactly
1/8 of all rows, zero halo, no collectives).

Precision: leaf-hop features (87.5% of bytes) stream as fp8 e3m4, inner
hops as bf16, weights bf16, PSUM accumulation f32. Measured end-to-end
rel err ~4e-3 vs the f32 reference.

Layout: channel-major [128ch, rows] so the matmul contraction dim is the
partition dim. Leaf columns are host-reordered PLANE-major per outer tile
(all sibling-0 cols, then sibling-1, ...) so the mean-aggregation is 8
accumulating matmuls with fully CONTIGUOUS 512-col rhs slabs -- strided
rhs runs 2-4x slower on the PE, contiguous hits 216ns/512col. The 1/8 is
folded into the aggregation weight. Inner aggregations (natural order,
sibling-adjacent) use the DVE reduce (4.4us/tile, dtype-independent); a
few leaf sub-tiles are offloaded to DVE via packed plane-adds to balance
engines. Only x is streamed from HBM (~43 MB/core).
"""

import os
import numpy as np

# ---------------------------------------------------------------- constants
N_CORES = 8
C = 128                       # channels
B = 4096                      # seeds
S = B // N_CORES              # 512 seeds per core
BLK = [512, 4096, 32768, 262144]          # per-core rows per hop
OFF = [0, 4096, 36864, 299008]            # global start row of each hop block
NIN = BLK[0] + BLK[1] + BLK[2]            # 37376 inner rows (hop 0-2)
NLEAF = BLK[3]                            # 262144 leaf rows
NPAR1 = BLK[0] + BLK[1]                   # 4608 layer-1 parents
PT = 512                                  # parents per PSUM tile
LB = 8 * BLK[1]                           # 32768 leaf rows per outer tile
N_FULL = 2396160
E_FULL = 2392064
OUT_ROWS = 36864

TRACE = os.environ.get("GNN_TRACE", "0") == "1"
# number of leaf sub-tiles (of 64) aggregated on DVE instead of PE
NDVE = int(os.environ.get("GNN_NDVE", "8"))
LAST_RESULT = None

_BASS_CACHE = {}


def _leaf_dve_flags(ndve):
    # spread ndve True flags evenly over the 64 leaf sub-tiles
    return [(m * ndve) // 64 != ((m + 1) * ndve) // 64 for m in range(64)]


def _build_bass(ndve):
    import concourse.mybir as mybir
    from concourse import bacc
    from concourse.tile import TileContext

    bf16 = mybir.dt.bfloat16
    f8e3 = mybir.dt.float8e3
    f32 = mybir.dt.float32
    Relu = mybir.ActivationFunctionType.Relu
    AxX = mybir.AxisListType.X
    Add = mybir.AluOpType.add

    dve_flag = _leaf_dve_flags(ndve)

    nc = bacc.Bacc()
    xiT = nc.dram_tensor("xiT", [C, NIN], bf16, kind="ExternalInput")
    xlT = nc.dram_tensor("xlT", [C, NLEAF], f8e3, kind="ExternalInput")
    wconsts = nc.dram_tensor("wconsts", [C, 6 * C], bf16, kind="ExternalInput")
    bconsts = nc.dram_tensor("bconsts", [C, 3], f32, kind="ExternalInput")
    out = nc.dram_tensor("out", [C, NPAR1], f32, kind="ExternalOutput")
    WIDX = {k: i for i, k in
            enumerate(("w1a", "w1b", "w2a", "w2b", "w3a", "w3b"))}

    with TileContext(nc) as tc:
        with tc.tile_pool(name="const", bufs=1) as constp, \
             tc.tile_pool(name="keep", bufs=1) as keepp, \
             tc.tile_pool(name="cbuf", bufs=2) as cpool, \
             tc.tile_pool(name="dbuf", bufs=3) as dpool, \
             tc.tile_pool(name="hbuf", bufs=2) as hpool, \
             tc.tile_pool(name="obuf", bufs=2) as opool, \
             tc.tile_pool(name="aggbuf", bufs=3) as aggp, \
             tc.tile_pool(name="addbuf", bufs=8) as addp, \
             tc.tile_pool(name="ps", bufs=8, space="PSUM") as pp:

            wtile = constp.tile([C, 6 * C], bf16, name="wtile")
            nc.sync.dma_start(wtile[:, :], wconsts[:, :])
            btile = constp.tile([C, 3], f32, name="btile")
            nc.sync.dma_start(btile[:, :], bconsts[:, :])
            w = {k: wtile[:, C * i: C * (i + 1)] for k, i in WIDX.items()}
            bt = {f"b{i+1}": btile[:, i: i + 1] for i in range(3)}

            xA01 = keepp.tile([C, NPAR1], bf16, tag="xA01")
            nc.sync.dma_start(xA01[:, :], xiT[:, 0:NPAR1])
            h1self = keepp.tile([C, NPAR1], bf16, tag="h1self")
            h2sb = keepp.tile([C, NPAR1], bf16, tag="h2sb")

            def dve_reduce(children_ap, tag, name):
                # DVE group-reduce over sibling-adjacent natural order
                aggt = aggp.tile([C, PT], bf16, tag=tag, name=name)
                with nc.allow_low_precision(reason="8-term sibling sum"):
                    nc.vector.reduce_sum(
                        aggt[:, :],
                        children_ap.rearrange("c (p e) -> c p e", e=8),
                        axis=AxX)
                return aggt

            def agg_mms(psum, wa, wb, aggt, self_ap):
                nc.tensor.matmul(psum, w[wa], aggt[:, :],
                                 start=True, stop=False)
                nc.tensor.matmul(psum, w[wb], self_ap,
                                 start=False, stop=True)

            def dve_sage(psum, wa, wb, children_ap, self_ap):
                aggt = dve_reduce(children_ap, "agg", "aggt")
                agg_mms(psum, wa, wb, aggt, self_ap)

            def leaf_pe_sage(psum, D, u, self_ap):
                # 8 accumulating mms over contiguous plane slabs
                for e in range(8):
                    nc.tensor.matmul(
                        psum, w["w1a"],
                        D[:, BLK[1] * e + PT * u: BLK[1] * e + PT * (u + 1)],
                        start=(e == 0), stop=False)
                nc.tensor.matmul(psum, w["w1b"], self_ap,
                                 start=False, stop=True)

            def leaf_dve_sage(psum, D, u, self_ap):
                # packed plane-adds: 4x (fp8+fp8->bf16), then 2+1 bf16
                def sl(e):
                    return D[:, BLK[1] * e + PT * u: BLK[1] * e + PT * (u + 1)]
                with nc.allow_low_precision(reason="8-term sibling sum"):
                    t4 = [addp.tile([C, PT], bf16, tag="add", name=f"t4_{j}")
                          for j in range(4)]
                    for j in range(4):
                        nc.vector.tensor_tensor(
                            t4[j][:, :], sl(2 * j), sl(2 * j + 1), op=Add)
                    s0 = addp.tile([C, PT], bf16, tag="add", name="s0")
                    nc.vector.tensor_tensor(s0[:, :], t4[0][:, :],
                                            t4[1][:, :], op=Add)
                    s1 = addp.tile([C, PT], bf16, tag="add", name="s1")
                    nc.vector.tensor_tensor(s1[:, :], t4[2][:, :],
                                            t4[3][:, :], op=Add)
                    aggt = aggp.tile([C, PT], bf16, tag="agg", name="aggd")
                    nc.vector.tensor_tensor(aggt[:, :], s0[:, :],
                                            s1[:, :], op=Add)
                nc.tensor.matmul(psum, w["w1a"], aggt[:, :],
                                 start=True, stop=False)
                nc.tensor.matmul(psum, w["w1b"], self_ap,
                                 start=False, stop=True)

            def w3b_tile(t):
                # h2 rows [512t, 512(t+1)) have no in-edges: self term only
                psn = pp.tile([C, PT], f32, tag="ps", name=f"psn{t}")
                nc.tensor.matmul(psn, w["w3b"],
                                 h2sb[:, PT * t: PT * (t + 1)],
                                 start=True, stop=True)
                on = opool.tile([C, PT], f32, tag="o", name=f"on{t}")
                nc.scalar.activation(on[:, :], psn, Relu, bias=bt["b3"])
                nc.sync.dma_start(out[:, PT * t: PT * (t + 1)], on[:, :])

            # layer-0 tile 0 (seeds) early: only needs xA01; fills DVE
            # while the first Ct/D DMAs stream.
            ps0z = pp.tile([C, PT], f32, tag="ps", name="ps0z")
            dve_sage(ps0z, "w1a", "w1b", xA01[:, S:NPAR1], xA01[:, 0:S])
            nc.scalar.activation(h1self[:, 0:S], ps0z, Relu, bias=bt["b1"])

            # Software-pipelined main loop: tile t's DVE-dependent matmuls
            # (layer-1 of t-1, layer-0-inner of t) are emitted AFTER tile
            # t's leaf matmul burst so the PE never waits on a reduce.
            h1tmp_prev = None
            for t in range(1, 9):
                Ct = cpool.tile([C, 8 * PT], bf16, tag="C")
                nc.sync.dma_start(
                    Ct[:, :], xiT[:, S + 8 * PT * t: S + 8 * PT * (t + 1)])
                D = dpool.tile([C, LB], f8e3, tag="D")
                nc.sync.dma_start(D[:, :], xlT[:, LB * (t - 1): LB * t])

                # DVE queue: L1(t-1) reduce (input ready), L0(t) reduce
                if h1tmp_prev is not None:
                    agg1p = dve_reduce(h1tmp_prev[:, :], "agg1", f"a1_{t}")
                agg0 = dve_reduce(Ct[:, :], "agg0", f"a0_{t}")

                # PE queue: ready-first. w3b(t-2), then the leaf burst.
                if t >= 3:
                    w3b_tile(t - 2)

                h1tmp = hpool.tile([C, 8 * PT], bf16, tag="h1tmp")
                for u in range(8):
                    psu = pp.tile([C, PT], f32, tag="ps", name=f"psu{t}_{u}")
                    if dve_flag[8 * (t - 1) + u]:
                        leaf_dve_sage(psu, D, u, Ct[:, PT * u: PT * (u + 1)])
                    else:
                        leaf_pe_sage(psu, D, u, Ct[:, PT * u: PT * (u + 1)])
                    nc.scalar.activation(h1tmp[:, PT * u: PT * (u + 1)], psu,
                                         Relu, bias=bt["b1"])

                # layer-0 tile for parents [512t, 512(t+1)) (hop-1 nodes)
                ps0 = pp.tile([C, PT], f32, tag="ps", name=f"ps0_{t}")
                agg_mms(ps0, "w1a", "w1b", agg0,
                        xA01[:, PT * t: PT * (t + 1)])
                nc.scalar.activation(h1self[:, PT * t: PT * (t + 1)], ps0,
                                     Relu, bias=bt["b1"])

                # layer-1 tile for parents [512(t-1), 512t) -> h2
                if h1tmp_prev is not None:
                    ps1 = pp.tile([C, PT], f32, tag="ps", name=f"ps1_{t}")
                    agg_mms(ps1, "w2a", "w2b", agg1p,
                            h1self[:, PT * (t - 1): PT * t])
                    nc.scalar.activation(h2sb[:, PT * (t - 1): PT * t], ps1,
                                         Relu, bias=bt["b2"])
                h1tmp_prev = h1tmp

            # drain: layer-1 tile 8, then tile 0, layer 2, w3b tail
            agg1p = dve_reduce(h1tmp_prev[:, :], "agg1", "a1_9")
            ps1 = pp.tile([C, PT], f32, tag="ps", name="ps1_9")
            agg_mms(ps1, "w2a", "w2b", agg1p, h1self[:, 8 * PT: 9 * PT])
            nc.scalar.activation(h2sb[:, 8 * PT: 9 * PT], ps1,
                                 Relu, bias=bt["b2"])
            w3b_tile(7)

            # layer-1 tile 0: children h1[512:4608) = h1self slice
            ps1z = pp.tile([C, PT], f32, tag="ps", name="ps1z")
            dve_sage(ps1z, "w2a", "w2b", h1self[:, S:NPAR1], h1self[:, 0:S])
            nc.scalar.activation(h2sb[:, 0:S], ps1z, Relu, bias=bt["b2"])
            w3b_tile(8)

            # layer 2: parents [0, 512) aggregate h2[512:4608)
            ps2 = pp.tile([C, PT], f32, tag="ps", name="ps2")
            dve_sage(ps2, "w3a", "w3b", h2sb[:, S:NPAR1], h2sb[:, 0:S])
            o0 = opool.tile([C, PT], f32, tag="o", name="o0")
            nc.scalar.activation(o0[:, :], ps2, Relu, bias=bt["b3"])
            nc.sync.dma_start(out[:, 0:S], o0[:, :])

    nc.compile()
    return nc


def _get_bass(ndve):
    if ndve not in _BASS_CACHE:
        _BASS_CACHE[ndve] = _build_bass(ndve)
    return _BASS_CACHE[ndve]


def _edge_is_tree(edge):
    if edge.shape != (2, E_FULL):
        return False
    ar = np.arange(E_FULL, dtype=np.int64)
    return (np.array_equal(edge[0], (B + ar).astype(np.int32))
            and np.array_equal(edge[1], (ar // 8).astype(np.int32)))


def _fallback(x, edge, W1, b1, W2, b2, W3, b3):
    # General (structure-agnostic) CPU implementation; only used if the
    # inputs are not the fanout-8 tree this kernel is specialized for.
    sizes = [(N_FULL, E_FULL), (299008, 294912), (36864, 32768)]
    params = [(W1, b1), (W2, b2), (W3, b3)]
    x = x.astype(np.float32)
    for (n, e), (Wl, bl) in zip(sizes, params):
        src = edge[0, :e].astype(np.int64)
        dst = edge[1, :e].astype(np.int64)
        x = x[:n]
        agg = np.zeros((n, x.shape[1]), np.float32)
        np.add.at(agg, dst, x[src])
        deg = np.bincount(dst, minlength=n).astype(np.float32)
        agg /= np.maximum(deg, 1.0)[:, None]
        x = np.maximum(np.concatenate([agg, x], axis=1) @ Wl.T + bl, 0.0)
    return x


def kernel(**inputs):
    global LAST_RESULT
    x = np.asarray(inputs["x"])
    edge = np.asarray(inputs["edge"])
    W = [np.asarray(inputs[k], dtype=np.float32) for k in ("W1", "W2", "W3")]
    bias = [np.asarray(inputs[k], dtype=np.float32) for k in ("b1", "b2", "b3")]

    if x.shape != (N_FULL, C) or not _edge_is_tree(edge):
        return _fallback(x, edge, W[0], bias[0], W[1], bias[1], W[2], bias[2])

    import ml_dtypes
    from concourse.bass_utils import run_bass_kernel_spmd

    bf16 = ml_dtypes.bfloat16
    f8e3 = ml_dtypes.float8_e3m4
    x = np.ascontiguousarray(x, dtype=np.float32)

    wblocks = []
    for li in range(3):
        wblocks.append((W[li][:, :C] / 8.0).T)     # agg part, mean folded in
        wblocks.append(W[li][:, C:].T)             # self part
    wconsts = np.concatenate(wblocks, axis=1).astype(bf16)
    wconsts = np.ascontiguousarray(wconsts)
    bconsts = np.ascontiguousarray(np.stack(bias, axis=1))      # [128, 3] f32

    in_maps = []
    for c in range(N_CORES):
        xi = np.concatenate(
            [x[OFF[h] + BLK[h] * c: OFF[h] + BLK[h] * (c + 1)]
             for h in range(3)], axis=0)
        xiTc = np.ascontiguousarray(xi.T).astype(bf16, copy=False)
        xl = x[OFF[3] + BLK[3] * c: OFF[3] + BLK[3] * (c + 1)]
        # plane-major per outer tile: [8 tiles, 4096 parents, 8 sib, 128ch]
        # -> [8, 8 sib, 4096 parents, 128]
        xl = xl.reshape(8, 4096, 8, C).transpose(0, 2, 1, 3).reshape(-1, C)
        xlTc = np.ascontiguousarray(xl.T).astype(f8e3, copy=False)
        in_maps.append({"xiT": xiTc, "xlT": xlTc,
                        "wconsts": wconsts, "bconsts": bconsts})

    nc = _get_bass(NDVE)
    res = run_bass_kernel_spmd(nc, in_maps, list(range(N_CORES)), trace=TRACE)
    LAST_RESULT = res

    out = np.empty((OUT_ROWS, C), np.float32)
    for c in range(N_CORES):
        oc = np.asarray(res.results[c]["out"])
        out[S * c: S * (c + 1)] = oc[:, :S].T
        out[B + 8 * S * c: B + 8 * S * (c + 1)] = oc[:, S:].T
    return out
